# revision 17
# baseline (speedup 1.0000x reference)
"""Trainium2 Bass kernel for the CovidModel scenario forecaster.

Math: the reference's 365-day lax.scan linearizes exactly.
With s(tau) = a0(tau) + eps*a1(tau) (the combined covariate):
    a_v(tau) = delta_v * u(tau) * s(tau-1),  u = rt^(1/T)
=>  s(tau)   = s(tau-1) * K * u(tau),        K = delta0 + eps*delta1
a pure cumulative product (hardware tensor_tensor_scan).  The three
Poisson-PMF window convolutions (a->m->e->out) are linear filters, so
    out(b,t) = sum_d C3[d] * q(b, t-3-d) + warmup boundary terms
with q = s/K and C3 the tap-composition of pi_G*pi_X*pi_M weighted by
rho/delta per vax status.  The warmup boundary only touches t<=30; its
(64-feature x 30-day) matrix is folded on the host together with s0
(both are O(B*64) gathers over the tiny warmup tensors).

Device pipeline per 128-scenario tile (scenarios on partitions):
  DMA rt -> ACT ln -> ACT exp(scale,bias) -> DVE scan (cumprod along
  time) -> PE transpose (time onto partitions) -> PE banded matmuls
  -> DVE copy (+warmup add) -> DMA out.
All PE inputs are produced by DVE so each Matmult carries a single
sync wait (the fused fp32 weight-load slot only fits one).
Sharding: batch B=16384 split 8 ways, pure data parallel, no
collectives; parameter-derived constants are replicated.
"""

import numpy as np

import concourse.bacc as bacc
import concourse.bass as bass
import concourse.mybir as mybir
import concourse.tile as tile
from concourse.bass_utils import run_bass_kernel_spmd

# Problem constants (fixed by the nn.Module definition)
J = 10
T_SERIAL = 5.8
B = 16384
FORECAST = 365
N_CORES = 8
B_SHARD = B // N_CORES          # 2048
N_TILES = B_SHARD // 128        # 16
TPAD = 384                      # 365 padded to 3 x 128
NCST = 3 * FORECAST + 128       # packed consts: mband chunks + identity
F32 = mybir.dt.float32


def _make_constants(eps, delta, rho_M, rho_X, rho_G, pi_M, pi_X, pi_G):
    """Fold the tiny replicated parameters into matmul constants."""
    eps, delta, rho_M, rho_X, rho_G, pi_M, pi_X, pi_G = [
        np.asarray(a, np.float64)
        for a in (eps, delta, rho_M, rho_X, rho_G, pi_M, pi_X, pi_G)
    ]
    K = delta[0] + eps[0] * delta[1]
    invT = 1.0 / T_SERIAL

    C3 = np.zeros(3 * (J - 1) + 1)
    for v in range(2):
        W = np.convolve(np.convolve(pi_G[v], pi_X[v]), pi_M[v])
        C3 += rho_G[v] * rho_X[v] * rho_M[v] * delta[v] * W
    C3n = C3 / K

    mband = np.zeros((TPAD, FORECAST))
    r = np.arange(FORECAST)[:, None]
    c = np.arange(FORECAST)[None, :]
    d = c - r - 3
    mask = (d >= 0) & (d <= 27)
    mband[:FORECAST][mask] = C3n[d[mask]]

    bm = np.zeros((64, 30))
    for v in range(2):
        for D in range(10):            # warmup day 20+D, tau = D - 9
            tau = D - 9
            for t in range(1, 31):
                col = t - 1
                j = t - 1 - tau
                if 0 <= j <= 9:
                    bm[40 + 10 * v + D, col] += rho_G[v] * pi_G[v, j]
                acc = 0.0
                for jj in range(10):
                    k = t - 2 - jj - tau
                    if 0 <= k <= 9 and (t - 1 - jj) >= 1:
                        acc += pi_G[v, jj] * pi_X[v, k]
                bm[20 + 10 * v + D, col] += rho_G[v] * rho_X[v] * acc
                acc = 0.0
                for jj in range(10):
                    for k in range(10):
                        l = t - 3 - jj - k - tau
                        if (0 <= l <= 9 and (t - 1 - jj) >= 1
                                and (t - 2 - jj - k) >= 1):
                            acc += pi_G[v, jj] * pi_X[v, k] * pi_M[v, l]
                bm[10 * v + D, col] += rho_G[v] * rho_X[v] * rho_M[v] * acc

    return (float(eps[0]), float(invT), float(np.log(K)),
            mband.astype(np.float32), bm.astype(np.float32))


def _build_nc(invT, lnK):
    nc = bacc.Bacc()

    rt_d = nc.dram_tensor("rt", [B_SHARD, FORECAST], F32, kind="ExternalInput")
    s0_d = nc.dram_tensor("s0", [B_SHARD, 1], F32, kind="ExternalInput")
    wc_d = nc.dram_tensor("wc", [B_SHARD, 30], F32, kind="ExternalInput")
    # packed constants: [0:1095] mband (3 tau-chunks side by side),
    # [1095:1223] identity
    cst_d = nc.dram_tensor("cst", [128, NCST], F32, kind="ExternalInput")
    out_d = nc.dram_tensor("out", [B_SHARD, FORECAST], F32, kind="ExternalOutput")

    Exp = mybir.ActivationFunctionType.Exp
    Ln = mybir.ActivationFunctionType.Ln

    with tile.TileContext(nc) as tc:
        with (
            tc.tile_pool(name="consts", bufs=1) as consts,
            tc.tile_pool(name="rt", bufs=3) as rt_pool,
            tc.tile_pool(name="work", bufs=3) as work,
            tc.tile_pool(name="st", bufs=3) as st_pool,
            tc.tile_pool(name="small", bufs=3) as small,
            tc.tile_pool(name="outp", bufs=3) as out_pool,
            tc.tile_pool(name="stp", bufs=2, space=bass.MemorySpace.PSUM) as st_psum,
            tc.tile_pool(name="op", bufs=3, space=bass.MemorySpace.PSUM) as out_psum,
        ):
            # stage constants through DVE so PE consumers sync on one sem
            cst_ld = consts.tile([128, NCST], F32, tag="cst_ld")
            nc.sync.dma_start(cst_ld[:], cst_d[:])
            cst = consts.tile([128, NCST], F32, tag="cst")
            nc.vector.tensor_copy(cst[:], cst_ld[:])
            mb_sb = cst[:, 0:3 * FORECAST]
            ident = cst[:, 3 * FORECAST:]
            lnk_sb = consts.tile([128, 1], F32, tag="lnk")
            nc.gpsimd.memset(lnk_sb[:], float(lnK))

            for i in range(N_TILES):
                rows = slice(i * 128, (i + 1) * 128)

                rt_t = rt_pool.tile([128, TPAD], F32, tag="rt")
                nc.sync.dma_start(rt_t[:, :FORECAST], rt_d[rows, :])
                nc.gpsimd.memset(rt_t[:, FORECAST:], 1.0)
                s0_t = small.tile([128, 1], F32, tag="s0")
                nc.sync.dma_start(s0_t[:], s0_d[rows, :])
                wc_t = small.tile([128, 30], F32, tag="wc")
                nc.sync.dma_start(wc_t[:], wc_d[rows, :])

                # f = exp(invT * ln(rt) + lnK); pad cols give f = K (finite)
                lr_t = work.tile([128, TPAD], F32, tag="lr")
                nc.scalar.activation(lr_t[:], rt_t[:], Ln)
                f_t = work.tile([128, TPAD], F32, tag="f")
                nc.scalar.activation(
                    f_t[:], lr_t[:], Exp, bias=lnk_sb[:, 0:1], scale=float(invT))

                # s(tau) cumulative product along time, seeded with s0
                s_t = work.tile([128, TPAD], F32, tag="s")
                nc.vector.tensor_tensor_scan(
                    s_t[:], f_t[:], f_t[:], s0_t[:],
                    op0=mybir.AluOpType.mult, op1=mybir.AluOpType.bypass)

                # transpose the 3 time chunks onto partitions
                stp = st_psum.tile([128, TPAD], F32, tag="stp")
                for chunk in range(3):
                    cs = slice(chunk * 128, (chunk + 1) * 128)
                    nc.tensor.transpose(stp[:, cs], s_t[:, cs], ident[:])
                st_sb = st_pool.tile([128, TPAD], F32, tag="st")
                nc.vector.tensor_copy(st_sb[:], stp[:])

                # banded matmuls: out(b,t) = sum_tau sT(tau,b)*mband(tau,t)
                op = out_psum.tile([128, FORECAST], F32, tag="op")
                nc.tensor.matmul(
                    op[:], st_sb[:, 0:128], mb_sb[:, 0:FORECAST],
                    start=True, stop=False)
                # chunk 1: tau 129..256 -> t in [132,286] -> cols 131..285
                nc.tensor.matmul(
                    op[:, 131:286], st_sb[:, 128:256],
                    mb_sb[:, FORECAST + 131:FORECAST + 286],
                    start=False, stop=False)
                # chunk 2: tau 257..365 -> t in [260,365] -> cols 259..364
                nc.tensor.matmul(
                    op[:, 259:365], st_sb[:, 256:384],
                    mb_sb[:, 2 * FORECAST + 259:2 * FORECAST + 365],
                    start=False, stop=True)

                # out = psum (+ warmup contribution on the first 30 days)
                o_sb = out_pool.tile([128, FORECAST], F32, tag="o")
                nc.vector.tensor_add(o_sb[:, 0:30], op[:, 0:30], wc_t[:])
                nc.vector.tensor_copy(o_sb[:, 30:], op[:, 30:])
                nc.sync.dma_start(out_d[rows, :], o_sb[:])

    nc.compile()
    return nc


_CACHE = {}


def _prep(inputs):
    """Returns (nc, in_maps) for the given full-size inputs."""
    r_t = np.ascontiguousarray(np.asarray(inputs["r_t"], np.float32))
    wa = np.asarray(inputs["warmup_asymp"], np.float32)
    wm = np.asarray(inputs["warmup_mild"], np.float32)
    we = np.asarray(inputs["warmup_extreme"], np.float32)

    eps, invT, lnK, mband, bm = _make_constants(
        inputs["eps"], inputs["delta"], inputs["rho_M"], inputs["rho_X"],
        inputs["rho_G"], inputs["pi_M"], inputs["pi_X"], inputs["pi_G"])

    key = (round(lnK, 12), round(invT, 12))
    if key not in _CACHE:
        _CACHE[key] = _build_nc(invT, lnK)
    nc = _CACHE[key]

    # warmup features: last 10 days of each compartment, (B, 64)
    wfeat = np.zeros((B, 64), np.float32)
    for ci, arr in enumerate((wa, wm, we)):
        for v in range(2):
            wfeat[:, 20 * ci + 10 * v: 20 * ci + 10 * v + 10] = arr[v, :, 20:30]
    s0 = (wfeat[:, 9] + np.float32(eps) * wfeat[:, 19]).reshape(B, 1)
    wc = wfeat @ bm                      # (B, 30) warmup boundary terms

    cstpack = np.zeros((128, NCST), np.float32)
    cstpack[:, :3 * FORECAST] = (
        mband.reshape(3, 128, FORECAST).transpose(1, 0, 2).reshape(128, -1))
    cstpack[:, 3 * FORECAST:] = np.eye(128, dtype=np.float32)

    in_maps = []
    for c in range(N_CORES):
        rows = slice(c * B_SHARD, (c + 1) * B_SHARD)
        in_maps.append({
            "rt": r_t[rows],
            "s0": np.ascontiguousarray(s0[rows]),
            "wc": np.ascontiguousarray(wc[rows]),
            "cst": cstpack,
        })
    return nc, in_maps


def kernel(**inputs):
    nc, in_maps = _prep(inputs)
    res = run_bass_kernel_spmd(nc, in_maps, list(range(N_CORES)))
    return np.concatenate([res.results[c]["out"] for c in range(N_CORES)], axis=0)
